# revision 1
# baseline (speedup 1.0000x reference)
"""AttentionBlock Trainium2 kernel: 8-way batch-parallel over 8 NeuronCores.

Reference computation (per batch element b):
    tokens = x[b].reshape(C, N).T                  # [N, C], N=1024, C=512
    qkv    = tokens @ w_proj + b_proj              # [N, 3*512]
    per head h (8 heads, D=64):
        att  = softmax(q_h @ k_h.T / 8, axis=keys) # [N, N]
        res_h = att @ v_h                          # [N, 64]
    out = res @ w_out + b_out + tokens             # [N, C]
    return out.T.reshape(C, 32, 32)

Kernel strategy (per core, one batch element):
  - qk projection computed transposed: qkT = w_qk.T @ x  -> SBUF [d, tokens]
    (w_proj columns host-permuted so each head-pair's q/k occupy partition
    halves 0-63 / 64-127, enabling row-packed K=64 score matmuls)
  - scores computed transposed scT[j, i] = k.T @ q, exp on ScalarE from PSUM
  - v projection computed untransposed (v = x.T @ w_v) with a ones column
    appended per head; attn@v matmul then yields [d | sum] x tokens, so the
    softmax denominator rides the same accumulation (M=65)
  - normalize via DVE reciprocal + DMA partition-broadcast + DVE multiply
  - out projection outT = w_out.T @ resT gives the output directly in x
    layout; residual and bias fused on DVE
  All matmul operands bf16 (fp32 PSUM accumulation).
"""
import sys
sys.path.insert(0, '/opt/trn_rl_repo')

import numpy as np
import ml_dtypes
from contextlib import ExitStack

B, C, N = 8, 512, 1024
NH, D = 8, 64
INNER = NH * D  # 512
SCALE = D ** -0.5

bf16 = ml_dtypes.bfloat16

_cached_run = None
_cached_nc = None


# ---------------------------------------------------------------- bass kernel
def _build_nc():
    import concourse.bass as bass
    import concourse.tile as tile
    from concourse import bacc, mybir
    from concourse import library_config

    f32 = mybir.dt.float32
    b16 = mybir.dt.bfloat16
    ts = bass.ts

    nc = bacc.Bacc("TRN2", target_bir_lowering=False, debug=False)

    x_d = nc.dram_tensor("x", [C, N], f32, kind="ExternalInput").ap()
    xb_d = nc.dram_tensor("xb", [C, N], b16, kind="ExternalInput").ap()
    wqk_d = nc.dram_tensor("wqk", [C, 1024], b16, kind="ExternalInput").ap()
    bqk_d = nc.dram_tensor("bqk", [128, 8], f32, kind="ExternalInput").ap()
    wv_d = nc.dram_tensor("wv", [C, 512], b16, kind="ExternalInput").ap()
    bvb_d = nc.dram_tensor("bvb", [128, 512], f32, kind="ExternalInput").ap()
    wo_d = nc.dram_tensor("wo", [INNER, C], b16, kind="ExternalInput").ap()
    bo_d = nc.dram_tensor("bo", [128, 4], f32, kind="ExternalInput").ap()
    out_d = nc.dram_tensor("out", [C, N], f32, kind="ExternalOutput").ap()

    with tile.TileContext(nc) as tc, ExitStack() as ctx:
        sb = ctx.enter_context(tc.tile_pool(name="sb", bufs=1))
        upool = ctx.enter_context(tc.tile_pool(name="up", bufs=1))
        rpool = ctx.enter_context(tc.tile_pool(name="rp", bufs=1))

        # ---- persistent SBUF tensors
        x_sb = sb.tile([128, 4, N], f32)
        nc.sync.dma_start(x_sb[:], x_d.rearrange("(kc p) n -> p kc n", p=128))
        xb_sb = sb.tile([128, 4, N], b16)
        nc.sync.dma_start(xb_sb[:], xb_d.rearrange("(kc p) n -> p kc n", p=128))
        wqk_sb = sb.tile([128, 4, 1024], b16)
        nc.sync.dma_start(wqk_sb[:], wqk_d.rearrange("(kc p) j -> p kc j", p=128))
        bqk_sb = sb.tile([128, 8], f32)
        nc.sync.dma_start(bqk_sb[:], bqk_d[:])
        wv_sb = sb.tile([128, 4, 512], b16)
        nc.sync.dma_start(wv_sb[:], wv_d.rearrange("(kc p) j -> p kc j", p=128))
        bvb_sb = sb.tile([128, 512], f32)
        nc.sync.dma_start(bvb_sb[:], bvb_d[:])
        wo_sb = sb.tile([128, 4, 512], b16)
        nc.sync.dma_start(wo_sb[:], wo_d.rearrange("(kc p) c -> p kc c", p=128))
        bo_sb = sb.tile([128, 4], f32)
        nc.sync.dma_start(bo_sb[:], bo_d[:])

        qkT_sb = sb.tile([128, 8, N], b16)      # [inner%128, qk chunk, token]
        v_sb = sb.tile([128, 8, 8 * 65], b16)   # [token%128, tchunk, h*65+(d|one)]
        v4 = v_sb.rearrange("p t (h w) -> p t h w", w=65)
        resT_sb = sb.tile([128, 4, N], b16)     # [inner%128, pair, token]
        final_sb = sb.tile([128, 4, N], f32)    # [c%128, cchunk, token]

        nc.vector.memset(v4[:, :, :, 64], 1.0)  # ones column per head
        ones_sb = sb.tile([128, 64], b16)
        nc.vector.memset(ones_sb[:], 1.0)  # lhsT for recip partition-broadcast
        for cc in range(4):  # final = x + b_out (residual+bias prefill)
            nc.vector.tensor_scalar_add(
                final_sb[:, cc, :], x_sb[:, cc, :], bo_sb[:, cc, None])

        def scores_pair(t):
            """Row-packed score matmuls + exp for head pair t."""
            qc, kc = 2 * t, 2 * t + 1
            uA = upool.tile([128, 8, N], b16, tag="U", bufs=4, name=f"u{2*t}")
            uB = upool.tile([128, 8, N], b16, tag="U", bufs=4, name=f"u{2*t+1}")
            for jc in range(8):
                sA = scA.tile([128, 2, 512], f32, tag="scA", bufs=1, name=f"sA{t}_{jc}")
                sB = scB.tile([128, 2, 512], f32, tag="scB", bufs=1, name=f"sB{t}_{jc}")
                for ih in range(2):
                    nc.tensor.matmul(
                        sA[:, ih, :],
                        lhsT=qkT_sb[0:64, kc, ts(jc, 128)],
                        rhs=qkT_sb[0:64, qc, ts(ih, 512)],
                        start=True, stop=True)
                for ih in range(2):
                    nc.tensor.matmul(
                        sB[:, ih, :],
                        lhsT=qkT_sb[64:128, kc, ts(jc, 128)],
                        rhs=qkT_sb[64:128, qc, ts(ih, 512)],
                        start=True, stop=True)
                nc.scalar.activation(
                    uA[:, jc, :], sA.rearrange("p a b -> p (a b)"),
                    mybir.ActivationFunctionType.Exp)
                nc.scalar.activation(
                    uB[:, jc, :], sB.rearrange("p a b -> p (a b)"),
                    mybir.ActivationFunctionType.Exp)
            return uA, uB

        # ---- projections (qk transposed, v direct)
        with tc.tile_pool(name="pp", bufs=3, space="PSUM") as pp:
            def qk_chunk(m):
                ps = pp.tile([128, 2, 512], f32, tag="pp", name=f"qk{m}")
                for ih in range(2):
                    for kc in range(4):
                        nc.tensor.matmul(
                            ps[:, ih, :],
                            lhsT=wqk_sb[:, kc, ts(m, 128)],
                            rhs=xb_sb[:, kc, ts(ih, 512)],
                            start=(kc == 0), stop=(kc == 3))
                nc.vector.tensor_scalar_add(
                    qkT_sb[:, m, :], ps.rearrange("p a b -> p (a b)"),
                    bqk_sb[:, m, None])

            for m in range(8):
                qk_chunk(m)
            for c2 in range(4):
                ps = pp.tile([128, 2, 512], f32, tag="pp", name=f"v{c2}")
                for half in range(2):
                    tch = 2 * c2 + half
                    for kc in range(4):
                        nc.tensor.matmul(
                            ps[:, half, :],
                            lhsT=xb_sb[:, kc, ts(tch, 128)],
                            rhs=wv_sb[:, kc, :],
                            start=(kc == 0), stop=(kc == 3))
                for half in range(2):
                    nc.vector.tensor_add(
                        v4[:, 2 * c2 + half, :, 0:64],
                        ps[:, half, :].rearrange("p (h d) -> p h d", d=64),
                        bvb_sb.rearrange("p (h d) -> p h d", d=64))

        # ---- attention: per pair, scores+exp then value-accum + normalize
        with tc.tile_pool(name="scA", bufs=1, space="PSUM") as scA, \
             tc.tile_pool(name="scB", bufs=1, space="PSUM") as scB, \
             tc.tile_pool(name="rsp", bufs=1, space="PSUM") as rsp, \
             tc.tile_pool(name="bcp", bufs=1, space="PSUM") as bcp:
            for t in range(4):
                uA, uB = scores_pair(t)
                for half in range(2):
                    h = 2 * t + half
                    u = uA if half == 0 else uB
                    res = rsp.tile([65, 2, 512], f32, tag="res", bufs=1,
                                   name=f"res{h}")
                    for jc in range(8):
                        for ih in range(2):
                            nc.tensor.matmul(
                                res[:, ih, :],
                                lhsT=v_sb[:, jc, h * 65:h * 65 + 65],
                                rhs=u[:, jc, ts(ih, 512)],
                                start=(jc == 0), stop=(jc == 7))
                    # reciprocal of the sums row (partition 64), broadcast to
                    # partitions 0-63 via a K=1 ones matmul, then normalize
                    rbc = rpool.tile([128, N], b16, tag="rbc", bufs=2,
                                     name=f"rbc{h}")
                    with nc.allow_low_precision(
                            reason="bf16 softmax-denominator reciprocal"):
                        nc.vector.reciprocal(
                            rbc[64:65, :],
                            res[64:65].rearrange("p a b -> p (a b)"))
                    bc = bcp.tile([64, 2, 512], f32, tag="bc", bufs=1,
                                  name=f"bc{h}")
                    for ih in range(2):
                        nc.tensor.matmul(
                            bc[:, ih, :],
                            lhsT=ones_sb[64:65, :],
                            rhs=rbc[64:65, ts(ih, 512)],
                            start=True, stop=True)
                    bcs = rpool.tile([64, N], f32, tag="bcs", bufs=2,
                                     name=f"bcs{h}")
                    nc.vector.tensor_copy(
                        bcs[:], bc.rearrange("p a b -> p (a b)"))
                    if half == 0:
                        nc.vector.tensor_mul(
                            resT_sb[0:64, t, :],
                            res[0:64].rearrange("p a b -> p (a b)"),
                            bcs[:])
                    else:
                        tmp = rpool.tile([64, N], b16, tag="tmpod", bufs=2,
                                         name=f"tm{h}")
                        nc.vector.tensor_mul(
                            tmp[:],
                            res[0:64].rearrange("p a b -> p (a b)"),
                            bcs[:])
                        nc.sync.dma_start(resT_sb[64:128, t, :], tmp[:])

        # ---- output projection + residual
        with tc.tile_pool(name="op", bufs=3, space="PSUM") as op:
            for cc in range(4):
                ps = op.tile([128, 2, 512], f32, tag="op", name=f"o{cc}")
                for ih in range(2):
                    for kc in range(4):
                        nc.tensor.matmul(
                            ps[:, ih, :],
                            lhsT=wo_sb[:, kc, ts(cc, 128)],
                            rhs=resT_sb[:, kc, ts(ih, 512)],
                            start=(kc == 0), stop=(kc == 3))
                nc.vector.tensor_add(
                    final_sb[:, cc, :], ps.rearrange("p a b -> p (a b)"),
                    final_sb[:, cc, :])
                nc.sync.dma_start(
                    out_d.rearrange("(cc p) n -> p cc n", p=128)[:, cc, :],
                    final_sb[:, cc, :])

    nc.compile()
    return nc


# ------------------------------------------------------------- SPMD dispatch
def _make_spmd_fn(nc, n_cores):
    """bass NEFF runner over axon PJRT WITHOUT buffer donation (donation
    hangs the axon backend)."""
    import jax
    import jax.core
    from jax.sharding import Mesh, PartitionSpec
    from jax.experimental.shard_map import shard_map
    from concourse import mybir
    from concourse.bass2jax import _bass_exec_p, install_neuronx_cc_hook

    install_neuronx_cc_hook()

    partition_name = nc.partition_id_tensor.name if nc.partition_id_tensor else None
    in_names, out_names, out_avals = [], [], []
    for alloc in nc.m.functions[0].allocations:
        if not isinstance(alloc, mybir.MemoryLocationSet):
            continue
        name = alloc.memorylocations[0].name
        if alloc.kind == "ExternalInput":
            if name != partition_name:
                in_names.append(name)
        elif alloc.kind == "ExternalOutput":
            out_names.append(name)
            out_avals.append(jax.core.ShapedArray(
                tuple(alloc.tensor_shape), mybir.dt.np(alloc.dtype)))

    n_params = len(in_names)
    all_in_names = list(in_names) + list(out_names)
    if partition_name is not None:
        all_in_names.append(partition_name)
    zero_outs = [np.zeros(a.shape, a.dtype) for a in out_avals]

    def _body(*args):
        operands = list(args)
        if partition_name is not None:
            from concourse.bass2jax import partition_id_tensor
            operands.append(partition_id_tensor())
        return tuple(_bass_exec_p.bind(
            *operands,
            out_avals=tuple(out_avals),
            in_names=tuple(all_in_names),
            out_names=tuple(out_names),
            lowering_input_output_aliases=(),
            sim_require_finite=True,
            sim_require_nnan=True,
            nc=nc,
        ))

    devices = jax.devices()[:n_cores]
    mesh = Mesh(np.asarray(devices), ("core",))
    sharded = jax.jit(
        shard_map(_body, mesh=mesh,
                  in_specs=(PartitionSpec("core"),) * (n_params + len(out_names)),
                  out_specs=(PartitionSpec("core"),) * len(out_names),
                  check_rep=False),
        keep_unused=True)

    def run(in_maps):
        per_core = [[np.asarray(m[k]) for k in in_names] for m in in_maps]
        concat = [np.concatenate([per_core[c][i] for c in range(n_cores)], axis=0)
                  for i in range(n_params)]
        concat += [np.concatenate([z] * n_cores, axis=0) for z in zero_outs]
        outs = [np.asarray(o) for o in sharded(*concat)]
        results = []
        for c in range(n_cores):
            m = {}
            for i, name in enumerate(out_names):
                rows = out_avals[i].shape[0]
                m[name] = outs[i][c * rows:(c + 1) * rows]
            results.append(m)
        return results

    return run


# ------------------------------------------------------------------ host prep
def _prep_weights(w_proj, b_proj, w_out, b_out):
    # permuted qk columns: chunk m (128 cols): pair t=m//2; m even -> q, odd -> k
    perm = np.empty(1024, np.int64)
    scale = np.empty(1024, np.float32)
    for m in range(8):
        t, is_k = m // 2, m % 2
        for p in range(128):
            h = 2 * t + (1 if p >= 64 else 0)
            d = p % 64
            perm[m * 128 + p] = h * 192 + 64 * is_k + d
            scale[m * 128 + p] = 1.0 if is_k else SCALE
    wqk = (w_proj[:, perm] * scale[None, :]).astype(bf16)
    bqk = (b_proj[perm] * scale).astype(np.float32).reshape(8, 128).T.copy()

    vperm = np.array([(j // 64) * 192 + 128 + (j % 64) for j in range(512)],
                     np.int64)
    wv = w_proj[:, vperm].astype(bf16)
    bvb = np.broadcast_to(b_proj[vperm].astype(np.float32), (128, 512)).copy()

    wo = w_out.astype(bf16)
    bo = b_out.astype(np.float32).reshape(4, 128).T.copy()
    return wqk, bqk, wv, bvb, wo, bo


def kernel(x, w_proj, b_proj, w_out, b_out):
    global _cached_run
    x = np.asarray(x, np.float32)
    w_proj = np.asarray(w_proj, np.float32)
    b_proj = np.asarray(b_proj, np.float32)
    w_out = np.asarray(w_out, np.float32)
    b_out = np.asarray(b_out, np.float32)

    global _cached_nc
    if _cached_run is None:
        nc = _build_nc()
        _cached_nc = nc
        _cached_run = _make_spmd_fn(nc, B)

    wqk, bqk, wv, bvb, wo, bo = _prep_weights(w_proj, b_proj, w_out, b_out)
    in_maps = []
    for b in range(B):
        x2d = np.ascontiguousarray(x[b].reshape(C, N))
        in_maps.append(dict(
            x=x2d, xb=x2d.astype(bf16), wqk=wqk, bqk=bqk,
            wv=wv, bvb=bvb, wo=wo, bo=bo))

    res = _cached_run(in_maps)
    out = np.stack([res[b]["out"].reshape(C, 32, 32) for b in range(B)])
    return out.astype(np.float32)



# revision 12
# speedup vs baseline: 1.3040x; 1.3040x over previous
"""AttentionBlock Trainium2 kernel: 8-way batch-parallel over 8 NeuronCores.

Reference computation (per batch element b):
    tokens = x[b].reshape(C, N).T                  # [N, C], N=1024, C=512
    qkv    = tokens @ w_proj + b_proj              # [N, 3*512]
    per head h (8 heads, D=64):
        att  = softmax(q_h @ k_h.T / 8, axis=keys) # [N, N]
        res_h = att @ v_h                          # [N, 64]
    out = res @ w_out + b_out + tokens             # [N, C]
    return out.T.reshape(C, 32, 32)

Kernel strategy (per core, one batch element):
  - qk projection computed transposed: qkT = w_qk.T @ x  -> SBUF [d, tokens]
    (w_proj columns host-permuted so each head-pair's q/k occupy partition
    halves 0-63 / 64-127, enabling row-group-packed K=64 score matmuls)
  - scores computed transposed scT[j, i] = k.T @ q into one [128,4,512] PSUM
    tile per (pair, jc); ONE ScalarE exp op of [128, 2048] covers both heads
  - v projection computed untransposed (v = x.T @ w_v) with a ones column
    appended per head; attn@v matmul then yields [d | sum] x tokens, so the
    softmax denominator rides the same accumulation (M=65)
  - normalize: DVE reciprocal_approx_fast on the PSUM denominator row,
    GpSimd partition_broadcast to 64 partitions, DVE multiply
  - scores/exp/attn@v interleaved per jc chunk so the PE trails ScalarE with
    short gaps (stays HAM-warm) instead of long stalls
  - out projection outT = w_out.T @ resT gives the output directly in x
    layout; residual and bias fused on DVE
  All matmul operands bf16 (fp32 PSUM accumulation).
"""
import sys
sys.path.insert(0, '/opt/trn_rl_repo')

import numpy as np
import ml_dtypes
from contextlib import ExitStack

B, C, N = 8, 512, 1024
NH, D = 8, 64
INNER = NH * D  # 512
SCALE = D ** -0.5

bf16 = ml_dtypes.bfloat16

_cached_run = None
_cached_nc = None


# ---------------------------------------------------------------- bass kernel
def _build_nc(debug_dumps=False):
    import concourse.bass as bass
    import concourse.tile as tile
    from concourse import bacc, mybir
    from concourse import library_config

    f32 = mybir.dt.float32
    b16 = mybir.dt.bfloat16
    ts = bass.ts

    nc = bacc.Bacc("TRN2", target_bir_lowering=False, debug=False)
    if debug_dumps:
        qkT_dump = nc.dram_tensor("qkT_dump", [128, 8, N], b16,
                                  kind="ExternalOutput").ap()
        v_dump = nc.dram_tensor("v_dump", [128, 8, 8 * 65], b16,
                                kind="ExternalOutput").ap()
        u3A_dump = nc.dram_tensor("u3A_dump", [128, 8, N], b16,
                                  kind="ExternalOutput").ap()
        u3B_dump = nc.dram_tensor("u3B_dump", [128, 8, N], b16,
                                  kind="ExternalOutput").ap()
        resT_dump = nc.dram_tensor("resT_dump", [128, 4, N], b16,
                                   kind="ExternalOutput").ap()

    xb_d = nc.dram_tensor("xb", [C, N], b16, kind="ExternalInput").ap()
    x_d = nc.dram_tensor("x", [C, N], f32, kind="ExternalInput").ap()
    wqk_d = nc.dram_tensor("wqk", [C, 1024], b16, kind="ExternalInput").ap()
    bqk_d = nc.dram_tensor("bqk", [128, 8], f32, kind="ExternalInput").ap()
    wv_d = nc.dram_tensor("wv", [C, 512], b16, kind="ExternalInput").ap()
    bvb_d = nc.dram_tensor("bvb", [128, 512], f32, kind="ExternalInput").ap()
    wo_d = nc.dram_tensor("wo", [INNER, C], b16, kind="ExternalInput").ap()
    bo_d = nc.dram_tensor("bo", [128, 4], f32, kind="ExternalInput").ap()
    out_d = nc.dram_tensor("out", [C, N], f32, kind="ExternalOutput").ap()

    with tile.TileContext(nc) as tc, ExitStack() as ctx:
        sb = ctx.enter_context(tc.tile_pool(name="sb", bufs=1))
        upool = ctx.enter_context(tc.tile_pool(name="up", bufs=1))
        rpool = ctx.enter_context(tc.tile_pool(name="rp", bufs=1))

        # ---- persistent SBUF tensors (inputs split for fine-grained deps)
        xb_k = []
        for kc in range(4):
            t_ = sb.tile([128, N], b16, name=f"xbk{kc}")
            nc.sync.dma_start(
                t_[:], xb_d.rearrange("(kc p) n -> kc p n", p=128)[kc])
            xb_k.append(t_)
        wqk_k = []
        for kc in range(4):
            t_ = sb.tile([128, 1024], b16, name=f"wqkk{kc}")
            nc.sync.dma_start(
                t_[:], wqk_d.rearrange("(kc p) j -> kc p j", p=128)[kc])
            wqk_k.append(t_)
        wv_k = []
        for kc in range(4):
            t_ = sb.tile([128, 512], b16, name=f"wvk{kc}")
            nc.sync.dma_start(
                t_[:], wv_d.rearrange("(kc p) j -> kc p j", p=128)[kc])
            wv_k.append(t_)
        bqk_sb = sb.tile([128, 8], f32)
        nc.sync.dma_start(bqk_sb[:], bqk_d[:])
        bvb_sb = sb.tile([128, 512], f32)
        nc.sync.dma_start(bvb_sb[:], bvb_d[:])
        # late-needed tensors (emitted first but consumed only at the tail;
        # DMA engines are parallel so these overlap compute)
        wo_sb = sb.tile([128, 4, 512], b16)
        nc.sync.dma_start(wo_sb[:], wo_d.rearrange("(kc p) c -> p kc c", p=128))
        bo_sb = sb.tile([128, 4], f32)
        nc.sync.dma_start(bo_sb[:], bo_d[:])
        final_sb = sb.tile([128, 4, N], f32)    # [c%128, cchunk, token]
        for cc in range(4):
            nc.sync.dma_start(
                final_sb[:, cc, :],
                x_d.rearrange("(cc p) n -> cc p n", p=128)[cc])

        qkT_sb = sb.tile([128, 8, N], b16)      # [inner%128, qk chunk, token]
        v_sb = sb.tile([128, 8, 8 * 65], b16)   # [token%128, tchunk, h*65+(d|one)]
        v4 = v_sb.rearrange("p t (h w) -> p t h w", w=65)
        resT_sb = sb.tile([128, 4, N], b16)     # [inner%128, pair, token]

        nc.vector.memset(v4[:, :, :, 64], 1.0)  # ones column per head

        with tc.tile_pool(name="scA", bufs=1, space="PSUM") as scA, \
             tc.tile_pool(name="scB", bufs=1, space="PSUM") as scB:

            def qk_chunk(pool, m):
                ps = pool.tile([128, 2, 512], f32, tag="pp", name=f"qk{m}")
                for ih in range(2):
                    for kc in range(4):
                        nc.tensor.matmul(
                            ps[:, ih, :],
                            lhsT=wqk_k[kc][:, ts(m, 128)],
                            rhs=xb_k[kc][:, ts(ih, 512)],
                            start=(kc == 0), stop=(kc == 3))
                nc.vector.tensor_scalar_add(
                    qkT_sb[:, m, :], ps.rearrange("p a b -> p (a b)"),
                    bqk_sb[:, m, None])

            def v_chunk(pool, c2):
                ps = pool.tile([128, 2, 512], f32, tag="pp", name=f"v{c2}")
                for half in range(2):
                    tch = 2 * c2 + half
                    for kc in range(4):
                        nc.tensor.matmul(
                            ps[:, half, :],
                            lhsT=xb_k[kc][:, ts(tch, 128)],
                            rhs=wv_k[kc][:],
                            start=(kc == 0), stop=(kc == 3))
                for half in range(2):
                    nc.vector.tensor_add(
                        v4[:, 2 * c2 + half, :, 0:64],
                        ps[:, half, :].rearrange("p (h d) -> p h d", d=64),
                        bvb_sb.rearrange("p (h d) -> p h d", d=64))

            def scores_jc(t, jc, uAB):
                """Row-group-packed score matmuls (A/B halves issue
                interleaved so they overlap on disjoint PE row groups),
                then one [128,1024] exp per half. A/B alternate on ScalarE
                so the next jc's A-matmuls refill under the B-exp."""
                qc, kc = 2 * t, 2 * t + 1
                sA = scA.tile([128, 2, 512], f32, tag="scA", name=f"sA{t}_{jc}")
                sB = scB.tile([128, 2, 512], f32, tag="scB", name=f"sB{t}_{jc}")
                for ih in range(2):
                    nc.tensor.matmul(
                        sA[:, ih, :],
                        lhsT=qkT_sb[0:64, kc, ts(jc, 128)],
                        rhs=qkT_sb[0:64, qc, ts(ih, 512)],
                        start=True, stop=True)
                    nc.tensor.matmul(
                        sB[:, ih, :],
                        lhsT=qkT_sb[64:128, kc, ts(jc, 128)],
                        rhs=qkT_sb[64:128, qc, ts(ih, 512)],
                        start=True, stop=True)
                nc.scalar.activation(
                    uAB[0][:, jc, :], sA.rearrange("p a b -> p (a b)"),
                    mybir.ActivationFunctionType.Exp)
                nc.scalar.activation(
                    uAB[1][:, jc, :], sB.rearrange("p a b -> p (a b)"),
                    mybir.ActivationFunctionType.Exp)

            def attnv_jc(rsp, t, jc, uAB, res_pair):
                for half in range(2):
                    h = 2 * t + half
                    res = res_pair[half]
                    for ih in range(2):
                        nc.tensor.matmul(
                            res[:, ih, :],
                            lhsT=v_sb[:, jc, h * 65:h * 65 + 65],
                            rhs=uAB[half][:, jc, ts(ih, 512)],
                            start=(jc == 0), stop=(jc == 7))

            def alloc_res_pair(rsp, t):
                return [rsp.tile([65, 2, 512], f32, tag="res", bufs=2,
                                 name=f"res{2 * t + half}")
                        for half in range(2)]

            def normalize(t, res_pair):
                """res[0:64] * (1/res[64]) -> resT. The denominator row is
                copied out of PSUM, DMA-reshaped across 128 partitions so the
                plain (HW-validated) reciprocal runs on free-size 16 instead
                of 1024, reshaped back, then a stride-0 free-dim DMA
                replicates it across 64 partitions for the DVE multiply."""
                den_sb = rpool.tile([65, 2, N], f32, tag="den", bufs=2,
                                    name=f"den{t}")
                for half in range(2):
                    nc.vector.tensor_copy(
                        den_sb[64:65, half, :],
                        res_pair[half][64:65].rearrange("p a b -> p (a b)"))
                denR = rpool.tile([128, 2, 8], f32, tag="denR", bufs=2,
                                  name=f"denR{t}")
                for half in range(2):
                    nc.sync.dma_start(
                        denR[:, half, :],
                        den_sb[64:65, half, :]
                        .rearrange("o (p k) -> o p k", k=8))
                recR = rpool.tile([128, 2, 8], b16, tag="recR", bufs=2,
                                  name=f"recR{t}")
                with nc.allow_low_precision(
                        reason="bf16 softmax-denominator reciprocal"):
                    nc.vector.reciprocal(
                        recR.rearrange("p a b -> p (a b)"),
                        denR.rearrange("p a b -> p (a b)"))
                recB = rpool.tile([65, 2, N], b16, tag="recB", bufs=2,
                                  name=f"recB{t}")
                for half in range(2):
                    nc.sync.dma_start(
                        recB[64:65, half, :]
                        .rearrange("o (p k) -> o p k", k=8),
                        recR[:, half, :])
                rbc = rpool.tile([64, 2, 2, 512], b16, tag="rbc", bufs=2,
                                 name=f"rbc{t}")
                for half in range(2):
                    for ih in range(2):
                        nc.sync.dma_start(
                            rbc[:, half, ih, :],
                            recB[64:65, half, ts(ih, 512)]
                            .unsqueeze(1).broadcast_to((1, 64, 512)))
                tmpO = rpool.tile([64, N], b16, tag="tmpO", bufs=2,
                                  name=f"tmpO{t}")
                for ih in range(2):
                    nc.vector.tensor_mul(
                        resT_sb[0:64, t, ts(ih, 512)],
                        res_pair[0][0:64, ih, :], rbc[:, 0, ih, :])
                    nc.vector.tensor_mul(
                        tmpO[:, ts(ih, 512)],
                        res_pair[1][0:64, ih, :], rbc[:, 1, ih, :])
                nc.sync.dma_start(resT_sb[64:128, t, :], tmpO[:])

            def alloc_u(t):
                uA = upool.tile([128, 8, N], b16, tag="U", bufs=4,
                                name=f"u{2 * t}")
                uB = upool.tile([128, 8, N], b16, tag="U", bufs=4,
                                name=f"u{2 * t + 1}")
                return (uA, uB)

            u_0 = alloc_u(0)
            # ---- phase 1: projections, with pair-0 scores/exp interleaved
            with tc.tile_pool(name="pp", bufs=2, space="PSUM") as pp0:
                qk_chunk(pp0, 0)
                qk_chunk(pp0, 1)
                # pair-0 scores start as soon as qkT chunks 0,1 exist; the
                # remaining projection work keeps the PE busy while ScalarE
                # runs exp. Ordered by earliest consumer: qk2/3 (pair-1
                # scores), v (pair-0 attn@v), qk4-7 (pairs 2-3).
                rest = [("qk", 2), ("qk", 3), ("v", 0), ("v", 1), ("v", 2),
                        ("v", 3), ("qk", 4), ("qk", 5), ("qk", 6), ("qk", 7)]
                sched = [2, 2, 2, 1, 1, 1, 1, 0]
                pos = 0
                for jc in range(8):
                    scores_jc(0, jc, u_0)
                    for _ in range(sched[jc]):
                        kind, i = rest[pos]
                        pos += 1
                        (qk_chunk if kind == "qk" else v_chunk)(pp0, i)

            # ---- phase 2: attention pipeline
            with tc.tile_pool(name="rsp", bufs=1, space="PSUM") as rsp:
                res_pair = alloc_res_pair(rsp, 0)
                u_prev, res_prev = u_0, res_pair
                for t in range(1, 4):
                    u_t = alloc_u(t)
                    res_pair = alloc_res_pair(rsp, t)
                    for jc in range(8):
                        scores_jc(t, jc, u_t)
                        attnv_jc(rsp, t - 1, jc, u_prev, res_prev)
                    normalize(t - 1, res_prev)
                    u_prev, res_prev = u_t, res_pair
                for jc in range(8):
                    attnv_jc(rsp, 3, jc, u_prev, res_prev)
                normalize(3, res_prev)
                if debug_dumps:
                    nc.sync.dma_start(qkT_dump[:], qkT_sb[:])
                    nc.sync.dma_start(v_dump[:], v_sb[:])
                    nc.sync.dma_start(u3A_dump[:], u_prev[0][:])
                    nc.sync.dma_start(u3B_dump[:], u_prev[1][:])
                    nc.sync.dma_start(resT_dump[:], resT_sb[:])

        # ---- output projection + residual
        with tc.tile_pool(name="op", bufs=3, space="PSUM") as op:
            for cc in range(4):
                # residual+bias prefill (in-place on the DMA'd x)
                nc.vector.tensor_scalar_add(
                    final_sb[:, cc, :], final_sb[:, cc, :], bo_sb[:, cc, None])
                ps = op.tile([128, 2, 512], f32, tag="op", name=f"o{cc}")
                for ih in range(2):
                    for kc in range(4):
                        nc.tensor.matmul(
                            ps[:, ih, :],
                            lhsT=wo_sb[:, kc, ts(cc, 128)],
                            rhs=resT_sb[:, kc, ts(ih, 512)],
                            start=(kc == 0), stop=(kc == 3))
                nc.vector.tensor_add(
                    final_sb[:, cc, :], ps.rearrange("p a b -> p (a b)"),
                    final_sb[:, cc, :])
                nc.sync.dma_start(
                    out_d.rearrange("(cc p) n -> p cc n", p=128)[:, cc, :],
                    final_sb[:, cc, :])

    nc.compile()
    return nc


# ------------------------------------------------------------- SPMD dispatch
def _make_spmd_fn(nc, n_cores):
    """bass NEFF runner over axon PJRT WITHOUT buffer donation (donation
    hangs the axon backend)."""
    import jax
    import jax.core
    from jax.sharding import Mesh, PartitionSpec
    from jax.experimental.shard_map import shard_map
    from concourse import mybir
    from concourse.bass2jax import _bass_exec_p, install_neuronx_cc_hook

    install_neuronx_cc_hook()

    partition_name = nc.partition_id_tensor.name if nc.partition_id_tensor else None
    in_names, out_names, out_avals = [], [], []
    for alloc in nc.m.functions[0].allocations:
        if not isinstance(alloc, mybir.MemoryLocationSet):
            continue
        name = alloc.memorylocations[0].name
        if alloc.kind == "ExternalInput":
            if name != partition_name:
                in_names.append(name)
        elif alloc.kind == "ExternalOutput":
            out_names.append(name)
            out_avals.append(jax.core.ShapedArray(
                tuple(alloc.tensor_shape), mybir.dt.np(alloc.dtype)))

    n_params = len(in_names)
    all_in_names = list(in_names) + list(out_names)
    if partition_name is not None:
        all_in_names.append(partition_name)
    zero_outs = [np.zeros(a.shape, a.dtype) for a in out_avals]

    def _body(*args):
        operands = list(args)
        if partition_name is not None:
            from concourse.bass2jax import partition_id_tensor
            operands.append(partition_id_tensor())
        return tuple(_bass_exec_p.bind(
            *operands,
            out_avals=tuple(out_avals),
            in_names=tuple(all_in_names),
            out_names=tuple(out_names),
            lowering_input_output_aliases=(),
            sim_require_finite=True,
            sim_require_nnan=True,
            nc=nc,
        ))

    devices = jax.devices()[:n_cores]
    mesh = Mesh(np.asarray(devices), ("core",))
    sharded = jax.jit(
        shard_map(_body, mesh=mesh,
                  in_specs=(PartitionSpec("core"),) * (n_params + len(out_names)),
                  out_specs=(PartitionSpec("core"),) * len(out_names),
                  check_rep=False),
        keep_unused=True)

    def run(in_maps):
        per_core = [[np.asarray(m[k]) for k in in_names] for m in in_maps]
        concat = [np.concatenate([per_core[c][i] for c in range(n_cores)], axis=0)
                  for i in range(n_params)]
        concat += [np.concatenate([z] * n_cores, axis=0) for z in zero_outs]
        outs = [np.asarray(o) for o in sharded(*concat)]
        results = []
        for c in range(n_cores):
            m = {}
            for i, name in enumerate(out_names):
                rows = out_avals[i].shape[0]
                m[name] = outs[i][c * rows:(c + 1) * rows]
            results.append(m)
        return results

    return run


# ------------------------------------------------------------------ host prep
def _prep_weights(w_proj, b_proj, w_out, b_out):
    # permuted qk columns: chunk m (128 cols): pair t=m//2; m even -> q, odd -> k
    perm = np.empty(1024, np.int64)
    scale = np.empty(1024, np.float32)
    for m in range(8):
        t, is_k = m // 2, m % 2
        for p in range(128):
            h = 2 * t + (1 if p >= 64 else 0)
            d = p % 64
            perm[m * 128 + p] = h * 192 + 64 * is_k + d
            scale[m * 128 + p] = 1.0 if is_k else SCALE
    wqk = (w_proj[:, perm] * scale[None, :]).astype(bf16)
    bqk = (b_proj[perm] * scale).astype(np.float32).reshape(8, 128).T.copy()

    vperm = np.array([(j // 64) * 192 + 128 + (j % 64) for j in range(512)],
                     np.int64)
    wv = w_proj[:, vperm].astype(bf16)
    bvb = np.broadcast_to(b_proj[vperm].astype(np.float32), (128, 512)).copy()

    wo = w_out.astype(bf16)
    bo = b_out.astype(np.float32).reshape(4, 128).T.copy()
    return wqk, bqk, wv, bvb, wo, bo


def kernel(x, w_proj, b_proj, w_out, b_out):
    global _cached_run
    x = np.asarray(x, np.float32)
    w_proj = np.asarray(w_proj, np.float32)
    b_proj = np.asarray(b_proj, np.float32)
    w_out = np.asarray(w_out, np.float32)
    b_out = np.asarray(b_out, np.float32)

    global _cached_nc
    if _cached_run is None:
        nc = _build_nc()
        _cached_nc = nc
        _cached_run = _make_spmd_fn(nc, B)

    wqk, bqk, wv, bvb, wo, bo = _prep_weights(w_proj, b_proj, w_out, b_out)
    in_maps = []
    for b in range(B):
        x2d = np.ascontiguousarray(x[b].reshape(C, N))
        in_maps.append(dict(
            x=x2d, xb=x2d.astype(bf16), wqk=wqk, bqk=bqk,
            wv=wv, bvb=bvb, wo=wo, bo=bo))

    res = _cached_run(in_maps)
    out = np.stack([res[b]["out"].reshape(C, 32, 32) for b in range(B)])
    return out.astype(np.float32)


# revision 15
# speedup vs baseline: 1.4723x; 1.1291x over previous
"""AttentionBlock Trainium2 kernel: 8-way batch-parallel over 8 NeuronCores.

Reference computation (per batch element b):
    tokens = x[b].reshape(C, N).T                  # [N, C], N=1024, C=512
    qkv    = tokens @ w_proj + b_proj              # [N, 3*512]
    per head h (8 heads, D=64):
        att  = softmax(q_h @ k_h.T / 8, axis=keys) # [N, N]
        res_h = att @ v_h                          # [N, 64]
    out = res @ w_out + b_out + tokens             # [N, C]
    return out.T.reshape(C, 32, 32)

Kernel strategy (per core, one batch element):
  - qk projection computed transposed: qkT = w_qk.T @ x  -> SBUF [d, tokens]
    (w_proj columns host-permuted so each head-pair's q/k occupy partition
    halves 0-63 / 64-127, enabling row-group-packed K=64 score matmuls)
  - scores computed transposed scT[j, i] = k.T @ q into one [128,4,512] PSUM
    tile per (pair, jc); ONE ScalarE exp op of [128, 2048] covers both heads
  - v projection computed untransposed (v = x.T @ w_v) with a ones column
    appended per head; attn@v matmul then yields [d | sum] x tokens, so the
    softmax denominator rides the same accumulation (M=65)
  - normalize: DVE reciprocal_approx_fast on the PSUM denominator row,
    GpSimd partition_broadcast to 64 partitions, DVE multiply
  - scores/exp/attn@v interleaved per jc chunk so the PE trails ScalarE with
    short gaps (stays HAM-warm) instead of long stalls
  - out projection outT = w_out.T @ resT gives the output directly in x
    layout; residual and bias fused on DVE
  All matmul operands bf16 (fp32 PSUM accumulation).
"""
import sys
sys.path.insert(0, '/opt/trn_rl_repo')

import numpy as np
import ml_dtypes
from contextlib import ExitStack

B, C, N = 8, 512, 1024
NH, D = 8, 64
INNER = NH * D  # 512
SCALE = D ** -0.5

bf16 = ml_dtypes.bfloat16

_cached_run = None
_cached_nc = None


# ---------------------------------------------------------------- bass kernel
def _build_nc(debug_dumps=False):
    import concourse.bass as bass
    import concourse.tile as tile
    from concourse import bacc, mybir
    from concourse import library_config

    f32 = mybir.dt.float32
    b16 = mybir.dt.bfloat16
    ts = bass.ts

    nc = bacc.Bacc("TRN2", target_bir_lowering=False, debug=False)
    if debug_dumps:
        qkT_dump = nc.dram_tensor("qkT_dump", [128, 8, N], b16,
                                  kind="ExternalOutput").ap()
        v_dump = nc.dram_tensor("v_dump", [128, 8, 8 * 65], b16,
                                kind="ExternalOutput").ap()
        u3A_dump = nc.dram_tensor("u3A_dump", [128, 8, N], b16,
                                  kind="ExternalOutput").ap()
        u3B_dump = nc.dram_tensor("u3B_dump", [128, 8, N], b16,
                                  kind="ExternalOutput").ap()
        resT_dump = nc.dram_tensor("resT_dump", [128, 4, N], b16,
                                   kind="ExternalOutput").ap()

    xb_d = nc.dram_tensor("xb", [C, N], b16, kind="ExternalInput").ap()
    x_d = nc.dram_tensor("x", [C, N], f32, kind="ExternalInput").ap()
    wqk_d = nc.dram_tensor("wqk", [C, 1024], b16, kind="ExternalInput").ap()
    bqk_d = nc.dram_tensor("bqk", [128, 8], f32, kind="ExternalInput").ap()
    wv_d = nc.dram_tensor("wv", [C, 512], b16, kind="ExternalInput").ap()
    bvb_d = nc.dram_tensor("bvb", [128, 512], f32, kind="ExternalInput").ap()
    wo_d = nc.dram_tensor("wo", [INNER, C], b16, kind="ExternalInput").ap()
    bo_d = nc.dram_tensor("bo", [128, 4], f32, kind="ExternalInput").ap()
    out_d = nc.dram_tensor("out", [C, N], f32, kind="ExternalOutput").ap()

    with tile.TileContext(nc) as tc, ExitStack() as ctx:
        sb = ctx.enter_context(tc.tile_pool(name="sb", bufs=1))
        upool = ctx.enter_context(tc.tile_pool(name="up", bufs=1))
        rpool = ctx.enter_context(tc.tile_pool(name="rp", bufs=1))

        # ---- persistent SBUF tensors (inputs split for fine-grained deps)
        xb_k = []
        for kc in range(4):
            t_ = sb.tile([128, N], b16, name=f"xbk{kc}")
            nc.sync.dma_start(
                t_[:], xb_d.rearrange("(kc p) n -> kc p n", p=128)[kc])
            xb_k.append(t_)
        wqk_k = []
        for kc in range(4):
            t_ = sb.tile([128, 1024], b16, name=f"wqkk{kc}")
            nc.sync.dma_start(
                t_[:], wqk_d.rearrange("(kc p) j -> kc p j", p=128)[kc])
            wqk_k.append(t_)
        wv_k = []
        for kc in range(4):
            t_ = sb.tile([128, 512], b16, name=f"wvk{kc}")
            nc.sync.dma_start(
                t_[:], wv_d.rearrange("(kc p) j -> kc p j", p=128)[kc])
            wv_k.append(t_)
        bqk_sb = sb.tile([128, 8], f32)
        nc.sync.dma_start(bqk_sb[:], bqk_d[:])
        bvb_sb = sb.tile([128, 512], f32)
        nc.sync.dma_start(bvb_sb[:], bvb_d[:])
        # late-needed tensors (emitted first but consumed only at the tail;
        # DMA engines are parallel so these overlap compute)
        wo_sb = sb.tile([128, 4, 512], b16)
        nc.sync.dma_start(wo_sb[:], wo_d.rearrange("(kc p) c -> p kc c", p=128))
        bo_sb = sb.tile([128, 4], f32)
        nc.sync.dma_start(bo_sb[:], bo_d[:])
        final_sb = sb.tile([128, 4, N], f32)    # [c%128, cchunk, token]
        for cc in range(4):
            nc.sync.dma_start(
                final_sb[:, cc, :],
                x_d.rearrange("(cc p) n -> cc p n", p=128)[cc])

        qkT_sb = sb.tile([128, 8, N], b16)      # [inner%128, qk chunk, token]
        v_sb = sb.tile([128, 8, 8 * 65], b16)   # [token%128, tchunk, h*65+(d|one)]
        v4 = v_sb.rearrange("p t (h w) -> p t h w", w=65)
        resT_sb = sb.tile([128, 4, N], b16)     # [inner%128, pair, token]

        nc.vector.memset(v4[:, :, :, 64], 1.0)  # ones column per head

        with tc.tile_pool(name="scA", bufs=1, space="PSUM") as scA, \
             tc.tile_pool(name="scB", bufs=1, space="PSUM") as scB:

            def qk_chunk(pool, m):
                ps = pool.tile([128, 2, 512], f32, tag="pp", name=f"qk{m}")
                for ih in range(2):
                    for kc in range(4):
                        nc.tensor.matmul(
                            ps[:, ih, :],
                            lhsT=wqk_k[kc][:, ts(m, 128)],
                            rhs=xb_k[kc][:, ts(ih, 512)],
                            start=(kc == 0), stop=(kc == 3))
                nc.vector.tensor_scalar_add(
                    qkT_sb[:, m, :], ps.rearrange("p a b -> p (a b)"),
                    bqk_sb[:, m, None])

            def v_chunk(pool, c2):
                ps = pool.tile([128, 2, 512], f32, tag="pp", name=f"v{c2}")
                for half in range(2):
                    tch = 2 * c2 + half
                    for kc in range(4):
                        nc.tensor.matmul(
                            ps[:, half, :],
                            lhsT=xb_k[kc][:, ts(tch, 128)],
                            rhs=wv_k[kc][:],
                            start=(kc == 0), stop=(kc == 3))
                for half in range(2):
                    nc.vector.tensor_add(
                        v4[:, 2 * c2 + half, :, 0:64],
                        ps[:, half, :].rearrange("p (h d) -> p h d", d=64),
                        bvb_sb.rearrange("p (h d) -> p h d", d=64))

            def scores_jc(t, jc, uAB):
                """Row-group-packed score matmuls (A/B halves issue
                interleaved so they overlap on disjoint PE row groups),
                then one [128,1024] exp per half. A/B alternate on ScalarE
                so the next jc's A-matmuls refill under the B-exp."""
                qc, kc = 2 * t, 2 * t + 1
                sA = scA.tile([128, 2, 512], f32, tag="scA", name=f"sA{t}_{jc}")
                sB = scB.tile([128, 2, 512], f32, tag="scB", name=f"sB{t}_{jc}")
                for ih in range(2):
                    nc.tensor.matmul(
                        sA[:, ih, :],
                        lhsT=qkT_sb[0:64, kc, ts(jc, 128)],
                        rhs=qkT_sb[0:64, qc, ts(ih, 512)],
                        start=True, stop=True)
                    nc.tensor.matmul(
                        sB[:, ih, :],
                        lhsT=qkT_sb[64:128, kc, ts(jc, 128)],
                        rhs=qkT_sb[64:128, qc, ts(ih, 512)],
                        start=True, stop=True)
                nc.scalar.activation(
                    uAB[0][:, jc, :], sA.rearrange("p a b -> p (a b)"),
                    mybir.ActivationFunctionType.Exp)
                nc.scalar.activation(
                    uAB[1][:, jc, :], sB.rearrange("p a b -> p (a b)"),
                    mybir.ActivationFunctionType.Exp)

            def attnv_jc(rsp, t, jc, uAB, res_pair):
                for half in range(2):
                    h = 2 * t + half
                    res = res_pair[half]
                    for ih in range(2):
                        nc.tensor.matmul(
                            res[:, ih, :],
                            lhsT=v_sb[:, jc, h * 65:h * 65 + 65],
                            rhs=uAB[half][:, jc, ts(ih, 512)],
                            start=(jc == 0), stop=(jc == 7))

            def alloc_res_pair(rsp, t):
                return [rsp.tile([65, 2, 512], f32, tag="res", bufs=2,
                                 name=f"res{2 * t + half}")
                        for half in range(2)]

            def normalize(t, res_pair):
                """res[0:64] * (1/res[64]) -> resT. Numerator and denominator
                are copied out of PSUM immediately (releasing the PSUM slots
                for the next pair's attn@v); the denominator is DMA-reshaped
                across 128 partitions so the plain reciprocal runs on
                free-size 16 instead of 1024, reshaped back, then a stride-0
                free-dim DMA replicates it across 64 partitions for an
                all-SBUF-bf16 DVE multiply."""
                raw = rpool.tile([64, 2, 2, 512], b16, tag="raw", bufs=2,
                                 name=f"raw{t}")
                den_sb = rpool.tile([65, 2, N], f32, tag="den", bufs=2,
                                    name=f"den{t}")
                for half in range(2):
                    nc.vector.tensor_copy(
                        raw[:, half, :, :].rearrange("p a b -> p (a b)"),
                        res_pair[half][0:64].rearrange("p a b -> p (a b)"))
                    nc.vector.tensor_copy(
                        den_sb[64:65, half, :],
                        res_pair[half][64:65].rearrange("p a b -> p (a b)"))
                denR = rpool.tile([128, 2, 8], f32, tag="denR", bufs=2,
                                  name=f"denR{t}")
                for half in range(2):
                    nc.sync.dma_start(
                        denR[:, half, :],
                        den_sb[64:65, half, :]
                        .rearrange("o (p k) -> o p k", k=8))
                recR = rpool.tile([128, 2, 8], b16, tag="recR", bufs=2,
                                  name=f"recR{t}")
                with nc.allow_low_precision(
                        reason="bf16 softmax-denominator reciprocal"):
                    nc.vector.reciprocal(
                        recR.rearrange("p a b -> p (a b)"),
                        denR.rearrange("p a b -> p (a b)"))
                recB = rpool.tile([65, 2, N], b16, tag="recB", bufs=2,
                                  name=f"recB{t}")
                for half in range(2):
                    nc.sync.dma_start(
                        recB[64:65, half, :]
                        .rearrange("o (p k) -> o p k", k=8),
                        recR[:, half, :])
                rbc = rpool.tile([64, 2, 2, 512], b16, tag="rbc", bufs=2,
                                 name=f"rbc{t}")
                for half in range(2):
                    for ih in range(2):
                        nc.sync.dma_start(
                            rbc[:, half, ih, :],
                            recB[64:65, half, ts(ih, 512)]
                            .unsqueeze(1).broadcast_to((1, 64, 512)))
                tmpO = rpool.tile([64, N], b16, tag="tmpO", bufs=2,
                                  name=f"tmpO{t}")
                for ih in range(2):
                    nc.vector.tensor_mul(
                        resT_sb[0:64, t, ts(ih, 512)],
                        raw[:, 0, ih, :], rbc[:, 0, ih, :])
                    nc.vector.tensor_mul(
                        tmpO[:, ts(ih, 512)],
                        raw[:, 1, ih, :], rbc[:, 1, ih, :])
                nc.sync.dma_start(resT_sb[64:128, t, :], tmpO[:])

            def alloc_u(t):
                uA = upool.tile([128, 8, N], b16, tag="U", bufs=4,
                                name=f"u{2 * t}")
                uB = upool.tile([128, 8, N], b16, tag="U", bufs=4,
                                name=f"u{2 * t + 1}")
                return (uA, uB)

            u_0 = alloc_u(0)
            # ---- phase 1: projections, with pair-0 scores/exp interleaved
            with tc.tile_pool(name="pp", bufs=2, space="PSUM") as pp0:
                qk_chunk(pp0, 0)
                qk_chunk(pp0, 1)
                # pair-0 scores start as soon as qkT chunks 0,1 exist; the
                # remaining projection work keeps the PE busy while ScalarE
                # runs exp. Ordered by earliest consumer: qk2/3 (pair-1
                # scores), v (pair-0 attn@v), qk4-7 (pairs 2-3).
                rest = [("qk", 2), ("qk", 3), ("v", 0), ("v", 1), ("v", 2),
                        ("v", 3), ("qk", 4), ("qk", 5), ("qk", 6), ("qk", 7)]
                # one ~1.7us projection chunk per jc step matches the ~2.1us
                # exp pace; the two leftovers (qk6/7, needed only by pair 3)
                # follow after
                for jc in range(8):
                    scores_jc(0, jc, u_0)
                    kind, i = rest[jc]
                    (qk_chunk if kind == "qk" else v_chunk)(pp0, i)
                for kind, i in rest[8:]:
                    (qk_chunk if kind == "qk" else v_chunk)(pp0, i)

            # ---- phase 2: attention pipeline
            with tc.tile_pool(name="rsp", bufs=1, space="PSUM") as rsp:
                res_pair = alloc_res_pair(rsp, 0)
                u_prev, res_prev = u_0, res_pair
                for t in range(1, 4):
                    u_t = alloc_u(t)
                    res_pair = alloc_res_pair(rsp, t)
                    for jc in range(8):
                        scores_jc(t, jc, u_t)
                        attnv_jc(rsp, t - 1, jc, u_prev, res_prev)
                    normalize(t - 1, res_prev)
                    u_prev, res_prev = u_t, res_pair
                for jc in range(8):
                    attnv_jc(rsp, 3, jc, u_prev, res_prev)
                normalize(3, res_prev)
                if debug_dumps:
                    nc.sync.dma_start(qkT_dump[:], qkT_sb[:])
                    nc.sync.dma_start(v_dump[:], v_sb[:])
                    nc.sync.dma_start(u3A_dump[:], u_prev[0][:])
                    nc.sync.dma_start(u3B_dump[:], u_prev[1][:])
                    nc.sync.dma_start(resT_dump[:], resT_sb[:])

        # ---- output projection + residual. kc=0..2 partial accumulation for
        # all 4 output chunks overlaps pair-3's normalize chain; only the
        # kc=3 matmuls + residual add + store remain in the tail.
        with tc.tile_pool(name="op", bufs=4, space="PSUM") as op:
            pstiles = []
            for cc in range(4):
                nc.vector.tensor_scalar_add(
                    final_sb[:, cc, :], final_sb[:, cc, :], bo_sb[:, cc, None])
                ps = op.tile([128, 2, 512], f32, tag="op", bufs=4,
                             name=f"o{cc}")
                pstiles.append(ps)
                for ih in range(2):
                    for kc in range(3):
                        nc.tensor.matmul(
                            ps[:, ih, :],
                            lhsT=wo_sb[:, kc, ts(cc, 128)],
                            rhs=resT_sb[:, kc, ts(ih, 512)],
                            start=(kc == 0), stop=False)
            for cc in range(4):
                ps = pstiles[cc]
                for ih in range(2):
                    nc.tensor.matmul(
                        ps[:, ih, :],
                        lhsT=wo_sb[:, 3, ts(cc, 128)],
                        rhs=resT_sb[:, 3, ts(ih, 512)],
                        start=False, stop=True)
                nc.vector.tensor_add(
                    final_sb[:, cc, :], ps.rearrange("p a b -> p (a b)"),
                    final_sb[:, cc, :])
                nc.sync.dma_start(
                    out_d.rearrange("(cc p) n -> p cc n", p=128)[:, cc, :],
                    final_sb[:, cc, :])

    nc.compile()
    return nc


# ------------------------------------------------------------- SPMD dispatch
def _make_spmd_fn(nc, n_cores):
    """bass NEFF runner over axon PJRT WITHOUT buffer donation (donation
    hangs the axon backend)."""
    import jax
    import jax.core
    from jax.sharding import Mesh, PartitionSpec
    from jax.experimental.shard_map import shard_map
    from concourse import mybir
    from concourse.bass2jax import _bass_exec_p, install_neuronx_cc_hook

    install_neuronx_cc_hook()

    partition_name = nc.partition_id_tensor.name if nc.partition_id_tensor else None
    in_names, out_names, out_avals = [], [], []
    for alloc in nc.m.functions[0].allocations:
        if not isinstance(alloc, mybir.MemoryLocationSet):
            continue
        name = alloc.memorylocations[0].name
        if alloc.kind == "ExternalInput":
            if name != partition_name:
                in_names.append(name)
        elif alloc.kind == "ExternalOutput":
            out_names.append(name)
            out_avals.append(jax.core.ShapedArray(
                tuple(alloc.tensor_shape), mybir.dt.np(alloc.dtype)))

    n_params = len(in_names)
    all_in_names = list(in_names) + list(out_names)
    if partition_name is not None:
        all_in_names.append(partition_name)
    zero_outs = [np.zeros(a.shape, a.dtype) for a in out_avals]

    def _body(*args):
        operands = list(args)
        if partition_name is not None:
            from concourse.bass2jax import partition_id_tensor
            operands.append(partition_id_tensor())
        return tuple(_bass_exec_p.bind(
            *operands,
            out_avals=tuple(out_avals),
            in_names=tuple(all_in_names),
            out_names=tuple(out_names),
            lowering_input_output_aliases=(),
            sim_require_finite=True,
            sim_require_nnan=True,
            nc=nc,
        ))

    devices = jax.devices()[:n_cores]
    mesh = Mesh(np.asarray(devices), ("core",))
    sharded = jax.jit(
        shard_map(_body, mesh=mesh,
                  in_specs=(PartitionSpec("core"),) * (n_params + len(out_names)),
                  out_specs=(PartitionSpec("core"),) * len(out_names),
                  check_rep=False),
        keep_unused=True)

    def run(in_maps):
        per_core = [[np.asarray(m[k]) for k in in_names] for m in in_maps]
        concat = [np.concatenate([per_core[c][i] for c in range(n_cores)], axis=0)
                  for i in range(n_params)]
        concat += [np.concatenate([z] * n_cores, axis=0) for z in zero_outs]
        outs = [np.asarray(o) for o in sharded(*concat)]
        results = []
        for c in range(n_cores):
            m = {}
            for i, name in enumerate(out_names):
                rows = out_avals[i].shape[0]
                m[name] = outs[i][c * rows:(c + 1) * rows]
            results.append(m)
        return results

    return run


# ------------------------------------------------------------------ host prep
def _prep_weights(w_proj, b_proj, w_out, b_out):
    # permuted qk columns: chunk m (128 cols): pair t=m//2; m even -> q, odd -> k
    perm = np.empty(1024, np.int64)
    scale = np.empty(1024, np.float32)
    for m in range(8):
        t, is_k = m // 2, m % 2
        for p in range(128):
            h = 2 * t + (1 if p >= 64 else 0)
            d = p % 64
            perm[m * 128 + p] = h * 192 + 64 * is_k + d
            scale[m * 128 + p] = 1.0 if is_k else SCALE
    wqk = (w_proj[:, perm] * scale[None, :]).astype(bf16)
    bqk = (b_proj[perm] * scale).astype(np.float32).reshape(8, 128).T.copy()

    vperm = np.array([(j // 64) * 192 + 128 + (j % 64) for j in range(512)],
                     np.int64)
    wv = w_proj[:, vperm].astype(bf16)
    bvb = np.broadcast_to(b_proj[vperm].astype(np.float32), (128, 512)).copy()

    wo = w_out.astype(bf16)
    bo = b_out.astype(np.float32).reshape(4, 128).T.copy()
    return wqk, bqk, wv, bvb, wo, bo


def kernel(x, w_proj, b_proj, w_out, b_out):
    global _cached_run
    x = np.asarray(x, np.float32)
    w_proj = np.asarray(w_proj, np.float32)
    b_proj = np.asarray(b_proj, np.float32)
    w_out = np.asarray(w_out, np.float32)
    b_out = np.asarray(b_out, np.float32)

    global _cached_nc
    if _cached_run is None:
        nc = _build_nc()
        _cached_nc = nc
        _cached_run = _make_spmd_fn(nc, B)

    wqk, bqk, wv, bvb, wo, bo = _prep_weights(w_proj, b_proj, w_out, b_out)
    in_maps = []
    for b in range(B):
        x2d = np.ascontiguousarray(x[b].reshape(C, N))
        in_maps.append(dict(
            x=x2d, xb=x2d.astype(bf16), wqk=wqk, bqk=bqk,
            wv=wv, bvb=bvb, wo=wo, bo=bo))

    res = _cached_run(in_maps)
    out = np.stack([res[b]["out"].reshape(C, 32, 32) for b in range(B)])
    return out.astype(np.float32)
